# revision 15
# baseline (speedup 1.0000x reference)
"""Trainium2 Bass kernel for nn_Net_86801289052267 (retrieval_knn).

Computes: out = one_hot(argmin_c ||means_c - mlp(x)||_2 + 5*t, 100)
where means_c are per-class mean features of mlp(mem_x) (100 exemplar rows).

Strategy (8 NeuronCores, data-parallel over the 16384-row batch):
  - The tiny exemplar path (100 rows, 0.005% of the FLOPs) runs on the host
    in float64; the device only needs V = -2*W3@means^T [2048, 5] because
    argmin_c ||means_c - pred||^2 = argmin_c (d_c + V[:,c].h2) -- affine in
    the last hidden layer h2, so layer 3 collapses to a [2048 -> 5] matmul.
  - Each core runs the 2-layer MLP on its 2048 rows entirely in bf16
    (fp32 PSUM accumulate): x is pre-transposed and bf16-cast on the host,
    W1/W2 are pre-packed into [m, p, k, c] strip layout so every DMA is a
    contiguous full-rate transfer and the device does zero transposes.
  - Weights stream through SBUF once per 1024-column batch half (42 MB of
    DMA per core vs 560 us of PE work -> fully hidden).
  - Device returns raw scores t = V^T h2 [5, 2048]; the host adds the d_c
    offsets, takes the argmin, and builds the one-hot. Rows whose score
    margin is below TAU are recomputed in float64 on the host (~5-10% of
    rows; bf16 device numerics are ~3e-3 rms on scores, flips live at
    margin < ~0.02), guaranteeing argmin parity with the fp32 reference.

Self-contained: hardcodes all shapes from the problem spec.
"""

import numpy as np
import ml_dtypes

BF = ml_dtypes.bfloat16

# Problem shapes (hardcoded per contract)
NS, DIN, DH, ND = 16384, 3072, 2048, 100
NCLS, NEX = 5, 20
NCORES = 8
ROWS = NS // NCORES        # 2048 x-rows per core
HALF = 1024                # batch columns per weight-streaming pass
KT1 = DIN // 128           # 24 k-tiles for layer 1
KT2 = DH // 128            # 16 k-tiles for layer 2/3
MT = DH // 128             # 16 feature strips
TAU = 0.05                 # host-refinement score-margin threshold

_CACHE = {}


def _to_bf16(a):
    """Fast fp32 -> bf16 with round-to-nearest-even (ml_dtypes astype is slow)."""
    u = np.ascontiguousarray(a, dtype=np.float32).view(np.uint32)
    out = ((u + 0x7FFF + ((u >> 16) & 1)) >> 16).astype(np.uint16)
    return out.view(BF)


def _build():
    """Build the 8-core SPMD Bass program. Returns the compiled Bass object."""
    import concourse.bacc as bacc
    import concourse.mybir as mybir
    import concourse.tile as tile
    from contextlib import ExitStack

    F32 = mybir.dt.float32
    BF16 = mybir.dt.bfloat16
    RELU = mybir.ActivationFunctionType.Relu

    nc = bacc.Bacc("TRN2", target_bir_lowering=False, debug=False,
                   num_devices=NCORES)

    xt = nc.dram_tensor("xt", [ROWS // 512, 128, KT1, 512], BF16,
                        kind="ExternalInput").ap()
    w1 = nc.dram_tensor("w1", [MT, 128, KT1, 128], BF16, kind="ExternalInput").ap()
    w2 = nc.dram_tensor("w2", [MT, 128, KT2, 128], BF16, kind="ExternalInput").ap()
    vt = nc.dram_tensor("vt", [128, KT2, NCLS], BF16, kind="ExternalInput").ap()
    b1t = nc.dram_tensor("b1t", [128, MT], F32, kind="ExternalInput").ap()
    b2t = nc.dram_tensor("b2t", [128, MT], F32, kind="ExternalInput").ap()
    tout = nc.dram_tensor("tout", [NCLS, ROWS], F32, kind="ExternalOutput").ap()

    with tile.TileContext(nc) as tc, ExitStack() as ctx:
        cpool = ctx.enter_context(tc.tile_pool(name="const", bufs=1))
        xtpool = ctx.enter_context(tc.tile_pool(name="xt", bufs=2))
        w1pool = ctx.enter_context(tc.tile_pool(name="w1", bufs=2))
        w2pool = ctx.enter_context(tc.tile_pool(name="w2", bufs=2))
        h1pool = ctx.enter_context(tc.tile_pool(name="h1", bufs=1))
        h2pool = ctx.enter_context(tc.tile_pool(name="h2", bufs=1))
        opool = ctx.enter_context(tc.tile_pool(name="o", bufs=2))
        mmps = ctx.enter_context(tc.tile_pool(name="mmps", bufs=6, space="PSUM"))
        l3ps = ctx.enter_context(tc.tile_pool(name="l3ps", bufs=2, space="PSUM"))

        # -- PE warm-up: dependency-free junk matmuls fill the initial DMA
        # window so the HAM clock gate is at 8/8 when real work arrives --
        wjunk = cpool.tile([128, 512], BF16, name="wjunk")
        nc.vector.memset(wjunk[:, :], 1.0)
        wps = mmps.tile([128, 512], F32, tag="mm", name="warm")
        for i in range(24):
            nc.tensor.matmul(wps[:, :], wjunk[:, 0:128], wjunk[:, :],
                             start=(i == 0), stop=(i == 23))

        vsb = cpool.tile([128, KT2, NCLS], BF16, name="vsb")
        b1sb = cpool.tile([128, MT], F32, name="b1sb")
        b2sb = cpool.tile([128, MT], F32, name="b2sb")

        for hb in range(2):
            base = hb * HALF
            # -- x^T chunk tiles, [din-part, k, 512] each, contiguous DMA --
            xcs = [xtpool.tile([128, KT1, 512], BF16, tag=f"xc_{c}",
                               name=f"xc_{c}_{hb}") for c in range(HALF // 512)]
            if hb == 0:
                # first chunk split across both HWDGE rings: shortest path
                # to the first real matmul
                nc.sync.dma_start(out=xcs[0][:, 0:KT1 // 2, :],
                                  in_=xt[0][:, 0:KT1 // 2, :])
                nc.scalar.dma_start(out=xcs[0][:, KT1 // 2:, :],
                                    in_=xt[0][:, KT1 // 2:, :])
                nc.sync.dma_start(out=xcs[1][:, :, :], in_=xt[1])
                # constants ride behind the critical x DMAs
                nc.sync.dma_start(out=vsb[:, :, :], in_=vt)
                nc.sync.dma_start(out=b1sb[:, :], in_=b1t)
                nc.sync.dma_start(out=b2sb[:, :], in_=b2t)
            else:
                for c in range(HALF // 512):
                    nc.sync.dma_start(out=xcs[c][:, :, :],
                                      in_=xt[hb * (HALF // 512) + c])

            # -- layer 1: h1T = relu(W1-strip.T @ xT + b1), bf16 out --
            h1s = [h1pool.tile([128, HALF], BF16, tag=f"h1_{m}",
                               name=f"h1_{m}_{hb}") for m in range(MT)]
            for m in range(MT):
                w1s = w1pool.tile([128, KT1, 128], BF16, tag="w1s",
                                  name=f"w1s{m}_{hb}")
                nc.scalar.dma_start(out=w1s[:, :, :], in_=w1[m])
                for c in range(HALF // 512):
                    ps = mmps.tile([128, 512], F32, tag="mm",
                                   name=f"p1_{hb}_{m}_{c}")
                    for k in range(KT1):
                        nc.tensor.matmul(ps[:, :], w1s[:, k, :],
                                         xcs[c][:, k, :],
                                         start=(k == 0), stop=(k == KT1 - 1))
                    nc.scalar.activation(h1s[m][:, 512 * c:512 * (c + 1)],
                                         ps[:, :], RELU,
                                         bias=b1sb[:, m:m + 1], scale=1.0)

            # -- layer 2: h2T = relu(W2-strip.T @ h1T + b2), bf16 out --
            h2s = [h2pool.tile([128, HALF], BF16, tag=f"h2_{m}",
                               name=f"h2_{m}_{hb}") for m in range(MT)]
            for m in range(MT):
                w2s = w2pool.tile([128, KT2, 128], BF16, tag="w2s",
                                  name=f"w2s{m}_{hb}")
                nc.scalar.dma_start(out=w2s[:, :, :], in_=w2[m])
                for c in range(HALF // 512):
                    ps = mmps.tile([128, 512], F32, tag="mm",
                                   name=f"p2_{hb}_{m}_{c}")
                    for k in range(KT2):
                        nc.tensor.matmul(ps[:, :], w2s[:, k, :],
                                         h1s[k][:, 512 * c:512 * (c + 1)],
                                         start=(k == 0), stop=(k == KT2 - 1))
                    nc.scalar.activation(h2s[m][:, 512 * c:512 * (c + 1)],
                                         ps[:, :], RELU,
                                         bias=b2sb[:, m:m + 1], scale=1.0)

            # -- layer 3: t = V.T @ h2T  [5, cols] --
            for c in range(HALF // 512):
                pt = l3ps.tile([NCLS, 512], F32, tag="l3", name=f"pt{hb}_{c}")
                for k in range(KT2):
                    nc.tensor.matmul(pt[:, :], vsb[:, k, :],
                                     h2s[k][:, 512 * c:512 * (c + 1)],
                                     start=(k == 0), stop=(k == KT2 - 1))
                tsb = opool.tile([NCLS, 512], F32, tag="tsb", name=f"tsb{hb}_{c}")
                nc.vector.tensor_copy(tsb[:, :], pt[:, :])
                nc.sync.dma_start(out=tout[:, base + 512 * c:base + 512 * (c + 1)],
                                  in_=tsb[:, :])

    nc.compile()
    return nc


def _host_means(mem_x, W1, b1, W2, b2, W3, b3):
    """Per-class mean exemplar features, float64 (100 rows -- tiny)."""
    W1d, b1d = W1.astype(np.float64), b1.astype(np.float64)
    W2d, b2d = W2.astype(np.float64), b2.astype(np.float64)
    W3d, b3d = W3.astype(np.float64), b3.astype(np.float64)
    nc_, ne_, din_ = mem_x.shape
    a = mem_x.reshape(nc_ * ne_, din_).astype(np.float64)
    h = np.maximum(a @ W1d + b1d, 0)
    h = np.maximum(h @ W2d + b2d, 0)
    feats = h @ W3d + b3d
    return feats.reshape(nc_, ne_, -1).mean(axis=1)  # [5, 100]


def _run(inputs, trace=False):
    """Prep/shard on host, execute on 8 cores, gather + refine."""
    from concourse import bass_utils

    x = np.ascontiguousarray(np.asarray(inputs["x"], dtype=np.float32))
    mem_x = np.asarray(inputs["mem_x"], dtype=np.float32)
    W1 = np.asarray(inputs["W1"], dtype=np.float32)
    b1 = np.asarray(inputs["b1"], dtype=np.float32)
    W2 = np.asarray(inputs["W2"], dtype=np.float32)
    b2 = np.asarray(inputs["b2"], dtype=np.float32)
    W3 = np.asarray(inputs["W3"], dtype=np.float32)
    b3 = np.asarray(inputs["b3"], dtype=np.float32)
    t_off = NCLS * int(np.asarray(inputs["t"]))

    if "nc" not in _CACHE:
        _CACHE["nc"] = _build()
    nc = _CACHE["nc"]

    # host-side exemplar path (float64) -> means, V, d
    means = _host_means(mem_x, W1, b1, W2, b2, W3, b3)       # [5, 100] f64
    V2 = -2.0 * (W3.astype(np.float64) @ means.T)            # [2048, 5] f64
    d = (means ** 2).sum(1) - 2.0 * means @ b3.astype(np.float64)  # [5] f64

    # pack device inputs (x: per-core, per-512-col-chunk, [part, k, col] so
    # every DMA reads one contiguous 24KB line per partition)
    xtp = np.ascontiguousarray(
        _to_bf16(x).reshape(NCORES, ROWS // 512, 512, KT1, 128)
        .transpose(0, 1, 4, 3, 2))
    w1p = np.ascontiguousarray(
        _to_bf16(W1).reshape(KT1, 128, MT, 128).transpose(2, 1, 0, 3))
    w2p = np.ascontiguousarray(
        _to_bf16(W2).reshape(KT2, 128, MT, 128).transpose(2, 1, 0, 3))
    vtp = np.ascontiguousarray(
        _to_bf16(V2.astype(np.float32)).reshape(KT2, 128, NCLS).transpose(1, 0, 2))
    b1p = np.ascontiguousarray(b1.reshape(MT, 128).T)
    b2p = np.ascontiguousarray(b2.reshape(MT, 128).T)

    in_maps = [{"xt": xtp[c], "w1": w1p, "w2": w2p, "vt": vtp,
                "b1t": b1p, "b2t": b2p} for c in range(NCORES)]

    res = bass_utils.run_bass_kernel_spmd(
        nc, in_maps, core_ids=list(range(NCORES)), trace=trace)

    tdev = np.concatenate(
        [res.results[c]["tout"].T for c in range(NCORES)], axis=0)  # [NS, 5]
    scores = tdev.astype(np.float64) + d[None, :]

    am = scores.argmin(axis=1)
    srt = np.sort(scores, axis=1)
    amb = (srt[:, 1] - srt[:, 0]) < TAU
    rows = np.nonzero(amb)[0]
    if rows.size:
        # exact float64 recompute of the ambiguous rows
        W1d, b1d = W1.astype(np.float64), b1.astype(np.float64)
        W2d, b2d = W2.astype(np.float64), b2.astype(np.float64)
        W3d, b3d = W3.astype(np.float64), b3.astype(np.float64)
        h = np.maximum(x[rows].astype(np.float64) @ W1d + b1d, 0)
        h = np.maximum(h @ W2d + b2d, 0)
        preds = h @ W3d + b3d
        d2 = ((means[None, :, :] - preds[:, None, :]) ** 2).sum(-1)
        am[rows] = d2.argmin(axis=1)

    out = np.zeros((NS, ND), dtype=np.float32)
    out[np.arange(NS), t_off + am] = 1.0
    return out, res, rows.size


def kernel(x, mem_x, W1, b1, W2, b2, W3, b3, t):
    out, _, _ = _run(dict(x=x, mem_x=mem_x, W1=W1, b1=b1, W2=W2, b2=b2,
                          W3=W3, b3=b3, t=t))
    return out
